# revision 32
# baseline (speedup 1.0000x reference)
"""Trainium2 Bass kernel for nn_MultiLatentAttention (B=2, S=2048, E=1024, H=16, P=64).

Math (exact reassociation of the reference):
  q = (x@WQ)@proj_w + proj_b          ->  x @ (WQ@proj_w) + proj_b
  attn1 - lam*attn2                   ->  q' @ k^T with q' = [s*q1, -s*lam*q2]
  (q'k^T) v                           ->  q' @ (k^T v)      (linear attention, no softmax)
  heads @ result_weight               ->  base @ W_eff,  W_eff[p,e] = sum_h (h+1)*RW[h*64+p, e]

Sharding: 8 cores, token-parallel for q/base/out (512 tokens each).  k^T v needs a
full-batch reduction; collectives cost ~6us on this runtime, so each core instead
computes k,v over its ENTIRE batch (x^T for the full batch is staged per-core, fp16,
with columns rotated so the core's own q-tokens are columns 0:512 -- k^T v is
permutation-invariant over tokens).

Schedule (sim 26026ns): DMA order [wkv+biases (Act queue), xt chunk 0, wq, xt
chunks 1-15, weff] -- 16 chunks of 128 tokens keep the PE backlog at stream end
minimal; weff goes last on the SP queue so it cannot delay the x stream.  The q
matmuls are scattered in 4 pieces after chunks 3-6 so the PE absorbs them in its
per-chunk slack and tracks the DMA stream.  M accumulates in its own PSUM bank,
all 16 matmuls emitted after the kv stream (a mid-stream wait stalls the in-order
PE sequencer).  PSUM->SBUF copies strictly alternate Act/DVE (gpsimd cannot read
PSUM on hardware).  Out tail: 4 token blocks of 2 matmuls + 2 copies + 1 DMA;
staging pool outp=4 so no block waits a prior DMA's 900ns completion semaphore;
pso=4 PSUM banks so the out matmuls never wait on a bank.  Output fp16 (host
converts to f32).
"""

import math

import numpy as np

import concourse.bass as bass
import concourse.tile as tile
from concourse import mybir
from concourse.bass_utils import run_bass_kernel_spmd

E = 1024
H = 16
P = 64        # per-head width (latent/H)
B = 2
S = 2048
N_CORES = 8
SH = 512      # q-tokens per core
KO = E // 128    # 8 contraction chunks
CH = 16          # xt DMA chunks (128 tokens each)
TPC = S // CH    # 128 tokens per DMA chunk
SUB = S // 128   # 16 compute sub-chunks of 128 tokens

WKV_C = KO * 2 * P      # 1024 kv-weight cols
BQ_C = 1                # q-bias column (64 partitions)
ROWS_C = 448            # bias/ones rows (partition 0 only)
WQ_C = KO * P           # 512 q-weight cols
HEAD_C = WKV_C + BQ_C + ROWS_C   # first-DMA region
WCOLS = HEAD_C + WQ_C

F16 = mybir.dt.float16
F32 = mybir.dt.float32


def _fix_excess_waits(nc, keep=1):
    """Split instructions with >keep sem waits (this walrus rejects multi-wait Drains)."""
    n_fixed = 0
    for f in nc.m.functions:
        for bb in f.blocks:
            insts = bb.instructions
            i = 0
            while i < len(insts):
                inst = insts[i]
                si = inst.sync_info
                waits = list(si.on_wait) if si is not None else []
                if len(waits) > keep:
                    excess, kept = waits[:-keep], waits[-keep:]
                    inst.sync_info = mybir.SyncInfo(on_wait=kept, on_update=list(si.on_update))
                    for k, w in enumerate(excess):
                        ev = mybir.InstEventSemaphore(
                            name=nc.get_next_instruction_name(),
                            engine=inst.engine, ins=[], outs=[],
                            sync_info=mybir.SyncInfo(on_wait=[w], on_update=[]),
                        )
                        nc.register_instruction(ev)
                        insts.insert(i + k, ev)
                    i += len(excess)
                    n_fixed += 1
                i += 1
    return n_fixed


DEFAULT_OPTS = dict(
    pskv_bufs=2, pso_bufs=4, quarter_copies=False, alt_dma=False,
    split_bt_mm=False, bt_copy_eng="vs",
)


def build_bass(**opts):
    o = {**DEFAULT_OPTS, **opts}
    nc = bass.Bass(num_devices=N_CORES, enable_partition_id=False)
    # xt: [128(ki), CH, KO, TPC] -- per-partition contiguous per chunk
    xt = nc.declare_dram_parameter("xt", [128, CH, KO, TPC], F16, isOutput=False)
    # wcomb: [wkv (KO*128) | rows(448, partition 0 only) | wq (KO*64)]
    wcomb = nc.declare_dram_parameter("wcomb", [128, WCOLS], F16, isOutput=False)
    weff = nc.declare_dram_parameter("weff", [P, E], F16, isOutput=False)
    out = nc.declare_dram_parameter("out", [SH, E], F16, isOutput=True)

    with tile.TileContext(nc) as tc:
        with (
            tc.tile_pool(name="singles", bufs=1) as singles,
            tc.tile_pool(name="xtp", bufs=CH) as xtp,
            tc.tile_pool(name="kvp", bufs=1) as kvp,
            tc.tile_pool(name="small", bufs=1) as small,
            tc.tile_pool(name="outp", bufs=4) as outp,
            tc.tile_pool(name="pskv", bufs=o["pskv_bufs"], space="PSUM") as pskv,
            tc.tile_pool(name="psacc", bufs=1, space="PSUM") as psacc,
            tc.tile_pool(name="psm", bufs=1, space="PSUM") as psm,
            tc.tile_pool(name="pso", bufs=o["pso_bufs"], space="PSUM") as pso,
        ):
            xt_tiles = [None] * CH

            def load_chunk(i):
                t = xtp.tile([128, KO, TPC], F16, tag="xt")
                nc.sync.dma_start(out=t, in_=xt[:, i])
                xt_tiles[i] = t

            wc_sb = singles.tile([128, WCOLS], F16)
            # kv weights + biases on Act's HWDGE queue (overlaps SP's issue of
            # chunk 0); then chunk 0; q weights; chunks 1-15; weff.
            nc.scalar.dma_start(out=wc_sb[:, 0:HEAD_C], in_=wcomb[:, 0:HEAD_C])
            load_chunk(0)
            nc.scalar.dma_start(out=wc_sb[:, HEAD_C:], in_=wcomb[:, HEAD_C:])
            for i in range(1, CH):
                load_chunk(i)
            # weff on the SP queue AFTER all xt chunks: issued from Act it
            # would enter the DMA engine queue mid-stream and delay xt.
            weff_sb = singles.tile([P, E], F16)
            nc.sync.dma_start(out=weff_sb, in_=weff[:, :])

            wkv_sb = wc_sb[:, 0:WKV_C].rearrange("p (ko c) -> p ko c", ko=KO)
            bq_sb = wc_sb[0:P, WKV_C:WKV_C + 1]
            rows_sb = wc_sb[0:1, WKV_C + BQ_C:HEAD_C]
            bkv_sb = rows_sb[:, 0:128]
            ones_sb = rows_sb[:, 192:448]
            wq_sb = wc_sb[:, HEAD_C:].rearrange("p (ko c) -> p ko c", ko=KO)

            kv_sb = kvp.tile([128, SUB, 2 * P], F16)
            ps_m = psm.tile([P, P], F32, name="ps_m")

            def kv_copy(j, ps):
                # strict Act/DVE alternation (gpsimd cannot read PSUM); odd
                # parity puts the final chunk-15 copy on DVE, idle by then.
                if j % 2 == 0:
                    nc.scalar.copy(out=kv_sb[:, j], in_=ps)
                else:
                    nc.vector.tensor_copy(out=kv_sb[:, j], in_=ps)

            def m_acc(j):
                # M += k_j^T v_j (all emitted after the kv stream: a mid-stream
                # sem wait on a kv copy stalls the in-order PE sequencer).
                nc.tensor.matmul(ps_m, kv_sb[:, j, 0:P], kv_sb[:, j, P:2 * P],
                                 start=(j == 0), stop=(j == SUB - 1),
                                 skip_group_check=True)

            def kv_chunk(j):
                ps = pskv.tile([128, 2 * P], F32, tag="kv")
                for ko in range(KO):
                    nc.tensor.matmul(ps, xt_tiles[j][:, ko],
                                     wkv_sb[:, ko], start=(ko == 0), stop=False)
                nc.tensor.matmul(ps, ones_sb[:, 0:128], bkv_sb, start=False, stop=True)
                kv_copy(j, ps)

            # ---- qT = wq^T @ xt[:, 0:512] -> [P, SH]; bias via activation.
            # Scattered in 4 token-column pieces after kv chunks 3..6 so the
            # PE absorbs the extra work early and tracks the DMA stream.
            ps_q = psacc.tile([P, SH], F32, tag="acc", name="ps_q")

            def q_piece(i2):
                for ko in range(KO):
                    nc.tensor.matmul(ps_q[:, i2 * TPC:(i2 + 1) * TPC],
                                     wq_sb[:, ko], xt_tiles[i2][:, ko],
                                     start=(ko == 0), stop=(ko == KO - 1),
                                     skip_group_check=True)

            for j in range(SUB):
                kv_chunk(j)
                if 3 <= j <= 6:
                    q_piece(j - 3)
                if j == 7:
                    qT_sb = small.tile([P, SH], F16)
                    nc.scalar.activation(out=qT_sb, in_=ps_q,
                                         func=mybir.ActivationFunctionType.Identity,
                                         bias=bq_sb)
            for j in range(SUB):
                m_acc(j)

            m_sb = small.tile([P, P], F16)
            nc.vector.tensor_copy(out=m_sb, in_=ps_m)

            # ---- baseT = M^T @ qT -> [P, SH] ----
            ps_bt = psacc.tile([P, SH], F32, tag="acc", name="ps_bt")
            if o["split_bt_mm"]:
                nc.tensor.matmul(ps_bt[:, 0:256], m_sb, qT_sb[:, 0:256],
                                 start=True, stop=True, skip_group_check=True)
                nc.tensor.matmul(ps_bt[:, 256:512], m_sb, qT_sb[:, 256:512],
                                 start=True, stop=True, skip_group_check=True)
            else:
                nc.tensor.matmul(ps_bt, m_sb, qT_sb, start=True, stop=True)
            bT_sb = small.tile([P, SH], F16)
            for i in range(4):
                seg = slice(i * 128, (i + 1) * 128)
                e0, e1 = o["bt_copy_eng"]
                eng = {"v": nc.vector.tensor_copy, "s": nc.scalar.copy,
                       "g": nc.gpsimd.tensor_copy}[e0 if i % 2 == 0 else e1]
                eng(out=bT_sb[:, seg], in_=ps_bt[:, seg])

            # ---- out = baseT^T @ weff (4 token blocks x 2 halves) ----
            rot = 0
            for i in range(SH // 128):
                o_sb = outp.tile([128, E], F16, tag="o")
                for h in range(2):
                    ps = pso.tile([128, 512], F32, tag="po")
                    nc.tensor.matmul(ps, bT_sb[:, i * 128:(i + 1) * 128],
                                     weff_sb[:, h * 512:(h + 1) * 512],
                                     start=True, stop=True)
                    # gpsimd cannot read PSUM on hardware: DVE/Act only.
                    nq = 2 if o["quarter_copies"] else 1
                    for q4 in range(nq):
                        w = 512 // nq
                        seg_o = slice(h * 512 + q4 * w, h * 512 + (q4 + 1) * w)
                        seg_p = slice(q4 * w, (q4 + 1) * w)
                        if rot % 2 == 0:
                            nc.vector.tensor_copy(out=o_sb[:, seg_o], in_=ps[:, seg_p])
                        else:
                            nc.scalar.copy(out=o_sb[:, seg_o], in_=ps[:, seg_p])
                        rot = (rot + 1) % 2
                # alternate SP (HWDGE) and gpsimd (SWDGE) issue queues so the
                # per-DMA data waits don't serialize on one sequencer.
                eng = nc.gpsimd if (o["alt_dma"] and i % 2 == 1) else nc.sync
                eng.dma_start(out=out[i * 128:(i + 1) * 128, :], in_=o_sb)

    _fix_excess_waits(nc)
    return nc


def _host_prep(x, WQ, WK, WV, result_weight, proj_w, proj_b,
               q1_vector, k1_vector, q2_vector, k2_vector, lambda_init):
    f64 = np.float64
    scale = 1.0 / math.sqrt(E // H)
    lam = (math.exp(float(np.dot(q1_vector.astype(f64), k1_vector.astype(f64))))
           - math.exp(float(np.dot(q2_vector.astype(f64), k2_vector.astype(f64))))
           + float(lambda_init[0]))

    wq_eff = WQ @ proj_w   # [E, P] f32
    wk_eff = WK @ proj_w
    wv_eff = WV @ proj_w

    d = np.concatenate([np.full(P // 2, scale), np.full(P // 2, -scale * lam)]).astype(np.float32)
    wq_s = wq_eff * d
    bq_s = proj_b * d

    mult = np.arange(1, H + 1, dtype=np.float32)
    weff = (result_weight.reshape(H, P, E) * mult[:, None, None]).sum(0, dtype=f64)  # [P, E]

    wkv = np.concatenate([wk_eff, wv_eff], axis=1)                  # [E, 2P]
    wkv16 = wkv.astype(np.float16).reshape(KO, 128, 2 * P).transpose(1, 0, 2)
    wq16 = wq_s.astype(np.float16).reshape(KO, 128, P).transpose(1, 0, 2)

    rows = np.zeros((ROWS_C,), np.float16)
    rows[0:P] = proj_b.astype(np.float16)
    rows[P:2 * P] = proj_b.astype(np.float16)
    rows[192:448] = 1.0
    wcomb16 = np.zeros((128, WCOLS), np.float16)
    wcomb16[:, 0:WKV_C] = wkv16.reshape(128, WKV_C)
    wcomb16[0:P, WKV_C] = bq_s.astype(np.float16)
    wcomb16[0, WKV_C + BQ_C:HEAD_C] = rows
    wcomb16[:, HEAD_C:] = wq16.reshape(128, WQ_C)
    weff16 = weff.astype(np.float16)

    in_maps = []
    for c in range(N_CORES):
        b = c // (N_CORES // B)
        s0 = (c % (N_CORES // B)) * SH
        xT = x[b].T                                    # [E, S] f32 view
        xrot = np.concatenate([xT[:, s0:], xT[:, :s0]], axis=1) if s0 else xT
        # [ki, CH, KO, TPC]: e = ko*128 + ki, t = i*TPC + tt
        xt16 = (xrot.astype(np.float16)
                .reshape(KO, 128, CH, TPC)     # [ko, ki, i, tt]
                .transpose(1, 2, 0, 3))        # [ki, i, ko, tt]
        in_maps.append({
            "xt": np.ascontiguousarray(xt16),
            "wcomb": wcomb16,
            "weff": np.ascontiguousarray(weff16),
        })
    return in_maps


_NC_CACHE = {}


def kernel(**inputs):
    inputs = {k: np.asarray(v) for k, v in inputs.items()}
    in_maps = _host_prep(**inputs)
    if "nc" not in _NC_CACHE:
        _NC_CACHE["nc"] = build_bass()
    res = run_bass_kernel_spmd(_NC_CACHE["nc"], in_maps, list(range(N_CORES)))
    out = np.empty((B, S, E), np.float32)
    for c in range(N_CORES):
        b = c // (N_CORES // B)
        s0 = (c % (N_CORES // B)) * SH
        out[b, s0:s0 + SH] = res.results[c]["out"].astype(np.float32)
    return out
